# revision 1
# baseline (speedup 1.0000x reference)
"""Multi-head attention (B=2, S=2048, E=1024, H=16, D=64) on 8 trn2 cores.

Sharding: head-parallel. Core c owns heads {2c, 2c+1} for both batches
(contiguous 128-wide column slice of x / of the attention output).
Each core computes q/k/v + attention for its 2 heads and a
contraction-sharded partial of the output projection (its 128 rows of
W_out^T); the host sums the 8 partials and adds the bias.

Layout trick: logits are computed TRANSPOSED (k on partitions, q free)
so that the exp() pass (ScalarE, PSUM->SBUF) directly produces P^T in
SBUF, which is exactly the operand layout the A@V matmul needs -- no
on-chip transpose or extra PSUM evacuation of the 16.7M-element P
matrix. The softmax denominators come for free from a ones-column
appended to V (stationary operand [v | 1], M=65): row 64 of the A@V
accumulator is sum_k exp(logits).

Matmul operands use float32r (fp32-width data the PE streams at 1
row/cycle vs 4 cycles/row for exact fp32; ~1e-4 rounding per matmul).
PSUM accumulation stays fp32.

Emission order interleaves the next batch's setup / previous batch's
output projection into the ACT-bound attention stream so no engine
sits idle between phases.
"""

import numpy as np

B, S, E, H, D = 2, 2048, 1024, 16, 64
NCORES = 8
SCALE = 0.125  # 1/sqrt(64)
NT = S // 128  # 16 seq tiles
QH = 2         # q halves per batch
QW = S // QH   # 1024

_PROG = None


def _build_program(loop_n=0, variant=""):
    import concourse.mybir as mybir
    import concourse.tile as tile
    from concourse import bacc
    from concourse._compat import get_trn_type

    fp32 = mybir.dt.float32
    f32r = mybir.dt.float32r
    AF = mybir.ActivationFunctionType
    Alu = mybir.AluOpType

    nc = bacc.Bacc(get_trn_type() or "TRN2", target_bir_lowering=False)
    xst = nc.dram_tensor("xst", [B, 128, S], f32r, kind="ExternalInput")
    bdq = nc.dram_tensor("bdq", [128, 128], f32r, kind="ExternalInput")
    bdk = nc.dram_tensor("bdk", [128, 128], f32r, kind="ExternalInput")
    bdv = nc.dram_tensor("bdv", [128, 128], f32r, kind="ExternalInput")
    wot = nc.dram_tensor("wot", [128, E], f32r, kind="ExternalInput")
    part = nc.dram_tensor("part", [B, S, E], fp32, kind="ExternalOutput")

    with tile.TileContext(nc) as tc:
        with (
            tc.tile_pool(name="consts", bufs=1) as consts,
            tc.tile_pool(name="xhT", bufs=2) as xhT_pool,
            tc.tile_pool(name="qT", bufs=2) as qT_pool,
            tc.tile_pool(name="kT", bufs=2) as kT_pool,
            tc.tile_pool(name="vaug", bufs=2) as vaug_pool,
            tc.tile_pool(name="pt", bufs=6) as pt_pool,
            tc.tile_pool(name="attnb", bufs=2) as attnb_pool,
            tc.tile_pool(name="small", bufs=8) as small_pool,
            tc.tile_pool(name="outsb", bufs=4) as out_pool,
            # 2 slots x [128,1024]f32 = 4 PSUM banks
            tc.tile_pool(name="psA", bufs=2, space="PSUM") as psA,
            # 2 slots x [65,1024]f32 = 4 PSUM banks (per-head AV accumulators)
            tc.tile_pool(name="psB", bufs=2, space="PSUM") as psB,
        ):
            bdq_sb = consts.tile([128, 128], f32r)
            nc.sync.dma_start(bdq_sb[:], bdq[:])
            bdk_sb = consts.tile([128, 128], f32r)
            nc.sync.dma_start(bdk_sb[:], bdk[:])
            bdv_sb = consts.tile([128, 128], f32r)
            nc.sync.dma_start(bdv_sb[:], bdv[:])
            wot_sb = consts.tile([128, E], f32r)
            nc.sync.dma_start(wot_sb[:], wot[:])
            ones64 = consts.tile([1, 64], f32r)
            nc.gpsimd.memset(ones64[:].bitcast(fp32), 1.0)

            xhT_t, qT_t, kT_t, vaug_t, attnb_t = {}, {}, {}, {}, {}

            def setup_steps(b):
                """x-transpose, q/k projections, v_aug build for batch b.

                Ordered so attention(b, 0) unblocks early: transposes for
                the first half, then q/k chunk 0, then v tiles 0-7, then
                the second half."""
                xhT = xhT_pool.tile([128, S], f32r, tag="xhT", name=f"xhT{b}")
                xhT_t[b] = xhT
                qT = qT_pool.tile([128, S], f32r, tag="qT", name=f"qT{b}")
                kT = kT_pool.tile([128, S], f32r, tag="kT", name=f"kT{b}")
                qT_t[b], kT_t[b] = qT, kT
                vaug = vaug_pool.tile([128, NT * 130], f32r, tag="vaug", name=f"va{b}")
                vaug_t[b] = vaug
                nc.gpsimd.memset(vaug[:].bitcast(fp32), 1.0)

                def qk_step(dst, w_sb, nm, c):
                    ps = psA.tile([128, 1024], fp32, tag="ps", name=f"{nm}p{b}_{c}")
                    for h2 in range(2):
                        nc.tensor.matmul(
                            ps[:, h2 * 512:(h2 + 1) * 512],
                            w_sb[:],
                            xhT[:, c * 1024 + h2 * 512: c * 1024 + (h2 + 1) * 512],
                        )
                    nc.vector.tensor_copy(dst[:, c * 1024:(c + 1) * 1024], ps[:])

                def v_step(st):
                    ps = psA.tile([128, 128], fp32, tag="ps", name=f"vp{b}_{st}")
                    nc.tensor.matmul(ps[:], xhT[:, st * 128:(st + 1) * 128], bdv_sb[:])
                    for i in range(2):
                        nc.vector.tensor_copy(
                            vaug[:, st * 130 + i * 65: st * 130 + i * 65 + 64],
                            ps[:, i * 64:(i + 1) * 64],
                        )

                nc.sync.dma_start(xhT[:, 0:QW], xst[b, :, 0:QW])
                nc.sync.dma_start(xhT[:, QW:S], xst[b, :, QW:S])
                qk_step(qT, bdq_sb, "q", 0)
                yield
                qk_step(kT, bdk_sb, "k", 0)
                yield
                for st in range(NT // 2):
                    v_step(st)
                    yield
                qk_step(qT, bdq_sb, "q", 1)
                yield
                qk_step(kT, bdk_sb, "k", 1)
                yield
                for st in range(NT // 2, NT):
                    v_step(st)
                    yield

            def final_steps(b, sts, tail=False):
                """Output-projection partial for seq tiles sts of batch b."""
                attnb = attnb_t[b]
                if "nofinal" in variant:
                    return
                for n, st in enumerate(sts):
                    o_sb = out_pool.tile([128, E], fp32, tag="o", name=f"o{b}_{st}")
                    for ec in range(2):
                        ps = psA.tile([128, 512], fp32, tag="ps", name=f"fp{b}_{st}_{ec}")
                        nc.tensor.matmul(
                            ps[:],
                            attnb[:, st * 128:(st + 1) * 128],
                            wot_sb[:, ec * 512:(ec + 1) * 512],
                        )
                        if tail and n % 2 == 0:
                            nc.scalar.copy(o_sb[:, ec * 512:(ec + 1) * 512], ps[:])
                        else:
                            nc.vector.tensor_copy(o_sb[:, ec * 512:(ec + 1) * 512], ps[:])
                        yield
                    if "nodma" not in variant:
                        nc.sync.dma_start(part[b, st * 128:(st + 1) * 128, :], o_sb[:])

            def pull(side, n=1):
                for _ in range(n):
                    for g in side:
                        try:
                            next(g)
                            break
                        except StopIteration:
                            continue

            def attention(b, qh, side):
                """logits^T -> exp -> AV for q-half qh of batch b."""
                qT, kT, vaug = qT_t[b], kT_t[b], vaug_t[b]
                if qh == 0:
                    attnb_t[b] = attnb_pool.tile(
                        [128, S], f32r, tag="attnb", name=f"at{b}"
                    )
                attnb = attnb_t[b]
                acc = [
                    psB.tile([65, QW], fp32, tag="acc", name=f"acc{b}_{qh}_{j}")
                    for j in range(2)
                ]
                def emit_av(u):
                    kt_, i_, pt_ = u
                    for h2 in range(QW // 512):
                        nc.tensor.matmul(
                            acc[i_][:, h2 * 512:(h2 + 1) * 512],
                            vaug[:, kt_ * 130 + i_ * 65: kt_ * 130 + (i_ + 1) * 65],
                            pt_[:, h2 * 512:(h2 + 1) * 512],
                            start=(kt_ == 0),
                            stop=(kt_ == NT - 1),
                        )

                # AV lags one unit behind logits/exp so the PE stream never
                # blocks the next exp behind an AV that waits on this exp.
                pending = None
                for kt in range(NT):
                    for i in range(2):
                        ps = psA.tile([128, QW], fp32, tag="ps", name=f"lg{b}{qh}{kt}{i}")
                        for h2 in range(QW // 512):
                            nc.tensor.matmul(
                                ps[:, h2 * 512:(h2 + 1) * 512],
                                kT[i * 64:(i + 1) * 64, kt * 128:(kt + 1) * 128],
                                qT[i * 64:(i + 1) * 64,
                                   qh * QW + h2 * 512: qh * QW + (h2 + 1) * 512],
                            )
                        ptile = pt_pool.tile([128, QW], f32r, tag="pt",
                                             name=f"pt{b}{qh}{kt}{i}")
                        nc.scalar.activation(ptile[:], ps[:], AF.Exp, scale=SCALE)
                        if pending is not None:
                            emit_av(pending)
                        pending = (kt, i, ptile)
                    pull(side)
                emit_av(pending)
                # normalize: attnb[i*64+d, q] = acc[d, q] / acc[64, q]
                # broadcast sums across 64 partitions via K=1 matmul, then
                # reciprocal, then multiply.
                for i in range(2):
                    sums_sb = small_pool.tile([1, QW], f32r, tag="sums",
                                              name=f"sm{b}{qh}{i}")
                    nc.vector.tensor_copy(sums_sb[:], acc[i][64:65, :])
                    bc_ps = psA.tile([64, QW], fp32, tag="ps", name=f"bc{b}{qh}{i}")
                    for h2 in range(QW // 512):
                        nc.tensor.matmul(
                            bc_ps[:, h2 * 512:(h2 + 1) * 512],
                            ones64[:],
                            sums_sb[0:1, h2 * 512:(h2 + 1) * 512],
                        )
                    inv_sb = small_pool.tile([64, QW], fp32, tag="inv",
                                             name=f"inv{b}{qh}{i}")
                    if "norecip" in variant:
                        nc.vector.tensor_copy(inv_sb[:], bc_ps[:])
                    else:
                        # ~51-ULP fast reciprocal (inputs are sums of
                        # positive exps -- no denorm/inf edge cases)
                        nc.vector.reciprocal_approx_fast(inv_sb[:], bc_ps[:])
                    nc.vector.tensor_tensor(
                        attnb[i * 64:(i + 1) * 64, qh * QW:(qh + 1) * QW],
                        acc[i][0:64, :],
                        inv_sb[:],
                        Alu.mult,
                    )

            # ---- schedule ----
            def emit_schedule():
                s0 = setup_steps(0)
                for _ in s0:
                    pass
                s1 = setup_steps(1)
                attention(0, 0, [s1])
                attention(0, 1, [s1])
                pull([s1], NT)  # drain any setup(1) leftovers
                f0 = final_steps(0, range(NT))
                attention(1, 0, [f0])
                f1a = final_steps(1, range(0, NT // 2))
                attention(1, 1, [f0, f1a])
                pull([f0, f1a], 2 * NT)
                for _ in final_steps(1, range(NT // 2, NT), tail=True):
                    pass

            if loop_n > 1:
                with tc.For_i(0, loop_n, 1, hint_engines=tuple(nc.engines)):
                    emit_schedule()
            else:
                emit_schedule()

    nc.compile()
    return nc


def _get_program():
    global _PROG
    if _PROG is None:
        import os
        _PROG = _build_program(int(os.environ.get("BASS_MHA_LOOP", "0")),
                               os.environ.get("BASS_MHA_VARIANT", ""))
    return _PROG


def make_in_maps(x, W_qkv, W_out):
    x = np.ascontiguousarray(np.asarray(x, dtype=np.float32))
    W_qkv = np.asarray(W_qkv, dtype=np.float32)
    W_out = np.asarray(W_out, dtype=np.float32)

    def bd(w):  # block_diag(w.T, w.T)
        out = np.zeros((128, 128), dtype=np.float32)
        out[0:64, 0:64] = w.T
        out[64:128, 64:128] = w.T
        return out

    bdq = bd(W_qkv[0:64])
    bdk = bd(W_qkv[64:128])
    bdv = bd(W_qkv[128:192])
    WoT = np.ascontiguousarray(W_out.T)
    in_maps = []
    for c in range(NCORES):
        in_maps.append({
            "xst": np.ascontiguousarray(x[:, :, c * 128:(c + 1) * 128].transpose(0, 2, 1)),
            "bdq": bdq,
            "bdk": bdk,
            "bdv": bdv,
            "wot": np.ascontiguousarray(WoT[c * 128:(c + 1) * 128, :]),
        })
    return in_maps


def kernel(x, W_qkv, W_out, b_out, _trace=False):
    from concourse import bass_utils

    nc = _get_program()
    in_maps = make_in_maps(x, W_qkv, W_out)
    res = bass_utils.run_bass_kernel_spmd(
        nc, in_maps, core_ids=list(range(NCORES)), trace=_trace
    )
    acc = np.zeros((B, S, E), dtype=np.float64)
    for r in res.results:
        acc += r["part"]
    acc += np.asarray(b_out, dtype=np.float64)
    out = acc.astype(np.float32)
    if _trace:
        return out, res
    return out

